# revision 61
# baseline (speedup 1.0000x reference)
"""Causal multi-head attention block (B=2, T=2048, D=1024, H=16) on 8 TRN2 cores.

Sharding: tensor-parallel over heads — each core owns 2 heads (128 cols of
w_attn's q/k/v blocks, 128 rows of w_proj) and produces a partial output
[B, T, D]; the host sums the 8 partials and adds the bias terms.

Per-core kernel. All matmul operands are bf16 (fp32 psum accumulation): bf16
runs 1 cycle/row at ANY output width, whereas fp32r drops to 4 cycles/row
below 256-wide — which crippled the 256 [*,128]-wide v matmuls (~40us/iter).

  phase 1 (QKV):  qT,kT [128f, B*T] = w^T @ x^T   (w stationary, xT moving)
                  v     [B*T, 128]  = x @ w_v     (xT tiles stationary)
                  v stored per (t-tile, head) as [128, 65] with a ones column
                  LAST — the ones column makes the AV matmul also produce the
                  softmax denominator in psum partition 64.
                  x t-chunks 0-1 come from persistent tiles prefetched at the
                  PREVIOUS iteration's tail (sync queue, ahead of the last out
                  DMAs) so the loop boundary never stalls the PE ~10us on x.
  phase 2 (attn): per (b, q-chunk of 512), both heads together:
                  sT [128k, 512q] = k @ qT (scores, transposed layout; causal
                  block-skipping: only k-tiles <= q-chunk get computed)
                  probs = exp(sT) (one flat [128,1024] ACT op for both heads),
                  straddle blocks get multiplicative {0,1} mask strips (DVE)
                  avT [65, 512] += [v|ones]^T @ probs  (row 64 = sum of exp)
                  normalize: DVE copy of the psum denominator row -> DMA shift
                  to partition 0 (gpsimd) -> reciprocal_approx_fast (DVE, ~5x
                  cheaper than exact reciprocal) -> partition_broadcast ->
                  aT = avT[0:64] * bc  -> a_sb (heads stacked via gpsimd DMA)
  phase 3 (proj): out [128t, 512e] = a_sb^T(128=2 heads) @ w_proj rows.
                  Emitted one q-chunk LATE (after the next chunk's attention)
                  so the PE rides over the ~4us normalize-chain latency; the
                  final chunk reads the per-head at tiles directly with
                  half-depth matmuls. Out rows are assembled to [128, 1024]
                  bf16 in SBUF and written with one DMA per t-tile.
"""
import numpy as np

import concourse.bass as bass
import concourse.mybir as mybir
import concourse.tile as tile
from concourse import bacc
from concourse.bass import ts, ds
from concourse.bass_utils import run_bass_kernel_spmd

F32 = mybir.dt.float32
F32R = mybir.dt.float32r
BF16 = mybir.dt.bfloat16

B, T, D = 2, 2048, 1024
H = 16
HD = D // H          # 64
N_CORES = 8
HPC = H // N_CORES   # heads per core = 2
CW = HPC * HD        # per-core head width = 128
TCH = 512            # q/t chunk width
NTCH = (B * T) // TCH   # 8 t-chunks over flattened (b, t)
NKT = T // 128       # 16 k-tiles per batch
NQC = T // TCH       # 4 q-chunks per batch


def build_program(reps: int = 1, phases: str = "123"):
    """Build the per-core Bass program (same program on all 8 cores).

    reps>1 wraps the body in a dynamic loop for wall-clock timing runs.
    """
    nc = bacc.Bacc("TRN2", target_bir_lowering=False, debug=False,
                   num_devices=N_CORES)

    xT = nc.dram_tensor("xT", [B, D, T], BF16, kind="ExternalInput")
    wq = nc.dram_tensor("wq", [D, CW], BF16, kind="ExternalInput")
    wk = nc.dram_tensor("wk", [D, CW], BF16, kind="ExternalInput")
    wv = nc.dram_tensor("wv", [D, CW], BF16, kind="ExternalInput")
    bq = nc.dram_tensor("bq", [CW], F32, kind="ExternalInput")   # pre-scaled /8
    bk = nc.dram_tensor("bk", [CW], F32, kind="ExternalInput")
    wp = nc.dram_tensor("wp", [CW, D], BF16, kind="ExternalInput")
    mask = nc.dram_tensor("mask", [128, NQC, TCH], BF16, kind="ExternalInput")
    out = nc.dram_tensor("out", [B, T, D], BF16, kind="ExternalOutput")

    with tile.TileContext(nc) as tc:
        with (
            tc.tile_pool(name="const", bufs=1) as const,
            tc.tile_pool(name="persist", bufs=1) as persist,
            tc.tile_pool(name="xt", bufs=4) as xt_pool,
            tc.tile_pool(name="probs", bufs=8) as probs_pool,
            tc.tile_pool(name="norm", bufs=3) as norm_pool,
            tc.tile_pool(name="osb", bufs=4) as osb_pool,
        ):
            # ---- constants / persistent state ----
            # issue-order matters at kernel start: the first QKV matmuls need
            # wq/wk and the first x d-tiles; everything else can trail.
            wq_sb = const.tile([128, D // 128, CW], BF16)
            wk_sb = const.tile([128, D // 128, CW], BF16)
            wv_sb = const.tile([128, D // 128, CW], BF16)
            wp_sb = const.tile([128, D], BF16)
            bq_sb = const.tile([128, 1], F32)
            bk_sb = const.tile([128, 1], F32)
            mask_sb = const.tile([128, NQC, TCH], BF16)
            # consts go on the gpsimd DMA queue so the first x chunks (sync
            # queue) land in parallel
            wq_r = wq.rearrange("(dt p) m -> p dt m", p=128)
            wk_r = wk.rearrange("(dt p) m -> p dt m", p=128)
            nc.gpsimd.dma_start(wq_sb[:, 0:1, :], wq_r[:, 0:1, :])
            nc.gpsimd.dma_start(wk_sb[:, 0:1, :], wk_r[:, 0:1, :])
            nc.gpsimd.dma_start(wq_sb[:, 1:, :], wq_r[:, 1:, :])
            nc.gpsimd.dma_start(wk_sb[:, 1:, :], wk_r[:, 1:, :])
            nc.gpsimd.dma_start(wv_sb[:], wv.rearrange("(dt p) m -> p dt m", p=128))
            nc.gpsimd.dma_start(bq_sb[:], bq[:, None])
            nc.gpsimd.dma_start(bk_sb[:], bk[:, None])
            nc.gpsimd.dma_start(wp_sb[:], wp[:, :])
            wp2_sb = const.tile([HD, HPC, D], BF16)
            nc.gpsimd.dma_start(wp2_sb[:], wp.rearrange("(h d) e -> d h e", h=HPC))
            nc.gpsimd.dma_start(mask_sb[:], mask[:, :, :])

            qT_sb = persist.tile([128, B * T], BF16)   # [2h*64, (b,t)]
            kT_sb = persist.tile([128, B * T], BF16)
            a_sb = persist.tile([128, B * T], BF16)    # normalized attn out ^T
            # persistent x tiles for t-chunks 0-1: loaded in the prologue and
            # re-loaded at each iteration's tail (the sync queue is busy with
            # out-DMAs there, but the transfers overlap the projection tail),
            # so the loop boundary never stalls the PE on the first x chunks.
            px = [persist.tile([128, D // 128, TCH], BF16, name=f"px{_c}")
                  for _c in range(2)]

            def emit_px_prefetch():
                for c in range(2):
                    b_i, qc = divmod(c, NQC)
                    nc.sync.dma_start(
                        px[c][:],
                        xT[b_i].rearrange("(dt p) t -> p dt t", p=128)[
                            :, :, ds(qc * TCH, TCH)])

            emit_px_prefetch()
            # v per t-tile & head: [v(64) | ones] columns — the trailing ones
            # column makes the AV matmul emit the softmax denominator in
            # psum partition 64 (64-aligned, so DVE ops can touch it).
            v_sb = persist.tile([128, B * T // 128, HPC, HD + 1], BF16)
            nc.vector.memset(v_sb[:, :, :, HD], 1.0)

            def body(_=None):
                # ================= phase 1: QKV projections =================
                with (
                    tc.tile_pool(name="ps_qk", bufs=4, space="PSUM") as ps_qk,
                    tc.tile_pool(name="ps_v", bufs=2, space="PSUM") as ps_v,
                ):
                    for c in range(NTCH):
                        b_i, qc = divmod(c, NQC)
                        if c < 2:
                            xt = px[c]   # prefetched at the previous tail
                        else:
                            xt = xt_pool.tile([128, D // 128, TCH], BF16)
                            xsrc = xT[b_i].rearrange(
                                "(dt p) t -> p dt t", p=128)[
                                :, :, ds(qc * TCH, TCH)]
                            # split per pair of d-tiles so the first matmul
                            # can start as soon as its slice lands
                            for g in range((D // 128) // 2):
                                nc.sync.dma_start(xt[:, ts(g, 2), :],
                                                  xsrc[:, ts(g, 2), :])
                        q_ps = ps_qk.tile([128, TCH], F32, tag="qk")
                        k_ps = ps_qk.tile([128, TCH], F32, tag="qk")
                        v_ps = ps_v.tile([128, TCH], F32)
                        # single-bank accumulation runs (all q, then all
                        # k) — avoids per-mm psum bank ping-pong; the q run
                        # still starts as the first DMA slices land
                        nd = D // 128
                        for dt in range(nd):
                            nc.tensor.matmul(q_ps[:], wq_sb[:, dt, :],
                                             xt[:, dt, :], start=dt == 0,
                                             stop=dt == nd - 1)
                        for dt in range(nd):
                            nc.tensor.matmul(k_ps[:], wk_sb[:, dt, :],
                                             xt[:, dt, :], start=dt == 0,
                                             stop=dt == nd - 1)
                        for s in range(TCH // 128):
                            for dt in range(nd):
                                nc.tensor.matmul(
                                    v_ps[:, ts(s, 128)],
                                    xt[:, dt, ts(s, 128)],
                                    wv_sb[:, dt, :],
                                    start=(dt == 0), stop=(dt == nd - 1))
                        # epilogues: q = psum/8 + bq/8 ; k = psum + bk
                        nc.vector.tensor_scalar(
                            qT_sb[:, ds(c * TCH, TCH)], q_ps[:], 0.125,
                            bq_sb[:],
                            mybir.AluOpType.mult, mybir.AluOpType.add)
                        nc.vector.tensor_scalar_add(
                            kT_sb[:, ds(c * TCH, TCH)], k_ps[:], bk_sb[:])
                        for s in range(TCH // 128):
                            tt = c * (TCH // 128) + s
                            nc.any.tensor_copy(
                                v_sb[:, tt, :, 0:HD],
                                v_ps[:, ds(s * 128, 128)].rearrange(
                                    "p (h d) -> p h d", h=HPC))

                if "2" not in phases:
                    return
                # ========== phases 2+3: attention + projection ==============
                # both heads processed together per (batch, q-chunk): their
                # 64-contraction score matmuls sit on partition bases 0/64 so
                # the PE runs them concurrently (row groups), and one ACT exp
                # covers both heads.
                with (
                    tc.tile_pool(name="ps_s", bufs=2, space="PSUM") as ps_s,
                    tc.tile_pool(name="ps_av", bufs=3, space="PSUM") as ps_av,
                    tc.tile_pool(name="ps_o", bufs=1, space="PSUM") as ps_o,
                ):
                    def emit_scores(b_i, qc, kt):
                        """scores + exp (+mask) for one k-block; returns the
                        probs tile for the AV matmuls."""
                        j = kt - 4 * qc   # >=0: straddles diagonal
                        f0 = max(j, 0) * 128  # cols f<128j fully masked
                        fsl = ds(f0, TCH - f0)
                        qcol = ds(b_i * T + qc * TCH, TCH)
                        ktcol = ds(b_i * T + kt * 128, 128)
                        sp_ps = ps_s.tile([128, HPC, TCH], F32)
                        for h in range(HPC):
                            hp = ds(h * HD, HD)
                            nc.tensor.matmul(
                                sp_ps[:, h, fsl],
                                kT_sb[hp, ktcol],
                                qT_sb[hp, qcol][:, fsl],
                                start=True, stop=True)
                        pp_sb = probs_pool.tile([128, HPC, TCH], BF16)
                        # full-width pairs get one flat contiguous [128, 1024]
                        # exp (the fast ACT shape); sliced straddle tiles get
                        # two contiguous 2D ops
                        if f0 == 0:
                            nc.scalar.activation(
                                pp_sb.rearrange("p h w -> p (h w)"),
                                sp_ps.rearrange("p h w -> p (h w)"),
                                mybir.ActivationFunctionType.Exp)
                        else:
                            for h in range(HPC):
                                nc.scalar.activation(
                                    pp_sb[:, h, fsl], sp_ps[:, h, fsl],
                                    mybir.ActivationFunctionType.Exp)
                        if j >= 0:
                            # only the 128-wide DIAGONAL q-subtile needs
                            # masking: for q-subcolumns beyond it the keep
                            # condition (p <= f - 128j) is always true, so
                            # multiplying there was a no-op. 128-wide ops cut
                            # DVE mask work ~2.5x and shrink the mask latency
                            # that gates the AV matmul behind the DVE backlog.
                            dsl = ds(f0, 128)
                            for h in range(HPC):
                                nc.vector.tensor_tensor(
                                    pp_sb[:, h, dsl], pp_sb[:, h, dsl],
                                    mask_sb[:, j, dsl],
                                    mybir.AluOpType.mult)
                        return pp_sb, fsl

                    def emit_av(b_i, qc, kt, avs, pp_sb, fsl):
                        nkt = 4 * qc + 4
                        for h in range(HPC):
                            nc.tensor.matmul(
                                avs[h][:, fsl],
                                v_sb[:, b_i * NKT + kt, h, :],
                                pp_sb[:, h, fsl],
                                start=(kt == 0), stop=(kt == nkt - 1))

                    def emit_attn(b_i, qc, last):
                        """attention for one q-chunk, then the softmax
                        normalize chain. Returns per-head at tiles."""
                        nkt = 4 * qc + 4   # causal: k-tiles 0..nkt-1
                        avs = [ps_av.tile([HD + 1, TCH], F32, tag="av",
                                          name=f"av{_h}")
                               for _h in range(HPC)]
                        for kt in range(nkt):
                            pp = emit_scores(b_i, qc, kt)
                            emit_av(b_i, qc, kt, avs, *pp)
                        # normalize rows 0..63 by the ones-row 64
                        qcol = ds(b_i * T + qc * TCH, TCH)
                        at_tiles = []
                        for h in range(HPC):
                            hp = ds(h * HD, HD)
                            av_ps = avs[h]
                            # denominator row: PSUM -> SBUF copy (DMA and the
                            # approx-recip bit trick can't touch PSUM), then
                            # DMA-shift to partition 0 for partition_broadcast
                            # (gpsimd queue: the pbcast consumer is next there
                            # anyway).
                            d64_sb = norm_pool.tile([HD + 1, TCH], F32,
                                                    tag="d64")
                            nc.vector.tensor_copy(
                                d64_sb[HD:HD + 1, :], av_ps[HD:HD + 1, :])
                            d0_sb = norm_pool.tile([1, TCH], F32, tag="d0")
                            nc.gpsimd.dma_start(d0_sb[:],
                                                d64_sb[HD:HD + 1, :])
                            r0_sb = norm_pool.tile([1, TCH], F32, tag="r0")
                            nc.vector.reciprocal_approx_fast(
                                r0_sb[:], d0_sb[:])
                            bc_sb = norm_pool.tile([HD, TCH], F32, tag="bc")
                            nc.gpsimd.partition_broadcast(bc_sb[:], r0_sb[:])
                            at_sb = norm_pool.tile([HD, TCH], BF16, tag="at")
                            nc.vector.tensor_tensor(
                                at_sb[:], av_ps[0:HD, :], bc_sb[:],
                                mybir.AluOpType.mult)
                            at_tiles.append(at_sb)
                            if not last:
                                # partition-shift into stacked-head layout
                                nc.gpsimd.dma_start(a_sb[hp, qcol], at_sb[:])
                        return at_tiles

                    def emit_proj(b_i, qc, at_tiles, last):
                        """projection for one q-chunk. Non-last chunks are
                        emitted one chunk late (after the NEXT chunk's
                        attention) so the PE rides over the normalize-chain
                        latency. The very last chunk reads the per-head at
                        tiles directly (half-depth matmuls) and borrows av-tag
                        psum slots to pipeline the copies."""
                        for ti, tt in enumerate(range(qc * 4, qc * 4 + 4)):
                            o_sb = osb_pool.tile([128, D], BF16)
                            for ec in range(D // TCH):
                                i = ti * (D // TCH) + ec
                                if last and i % 2 == 1:
                                    o_ps = ps_av.tile([128, TCH], F32,
                                                      tag="av", name="o_av")
                                else:
                                    o_ps = ps_o.tile([128, TCH], F32)
                                if last:
                                    tloc = ds((tt - qc * 4) * 128, 128)
                                    for h in range(HPC):
                                        nc.tensor.matmul(
                                            o_ps[:],
                                            at_tiles[h][:, tloc],
                                            wp2_sb[:, h, ts(ec, TCH)],
                                            start=(h == 0),
                                            stop=(h == HPC - 1))
                                else:
                                    nc.tensor.matmul(
                                        o_ps[:],
                                        a_sb[:, ds(b_i * T + tt * 128, 128)],
                                        wp_sb[:, ts(ec, TCH)],
                                        start=True, stop=True)
                                nc.any.tensor_copy(
                                    o_sb[:, ts(ec, TCH)], o_ps[:])
                            nc.sync.dma_start(
                                out[b_i, ts(tt, 128), :], o_sb[:])

                    pending = None
                    for b_i in range(B):
                        for qc in range(NQC):
                            last = (b_i == B - 1 and qc == NQC - 1)
                            at_tiles = emit_attn(b_i, qc, last)
                            if "3" not in phases:
                                continue
                            if b_i == B - 1 and qc == 0:
                                # re-load the first x chunks for the next
                                # iteration. Emitted HERE (mid phase 2, ~50us
                                # before the tail) so the 2MB of transfers
                                # never competes with the tail projections'
                                # out-DMAs — issued at the tail it delayed
                                # o_sb recycling and stalled the next
                                # iteration's PE ~6.5us through the weight
                                # shadow.
                                emit_px_prefetch()
                            if pending is not None:
                                emit_proj(*pending, last=False)
                            if last:
                                emit_proj(b_i, qc, at_tiles, last=True)
                            else:
                                pending = (b_i, qc, None)

            if reps == 1:
                body()
            else:
                with tc.For_i(0, reps, 1) as _i:
                    body(_i)

    nc.compile()
    return nc


def make_mask() -> np.ndarray:
    """Multiplicative mask strips for the 4 diagonal-straddling k-tiles of a
    512-wide q-chunk: strip j keeps (p <= f - 128*j)."""
    p = np.arange(128)[:, None]
    f = np.arange(TCH)[None, :]
    m = np.stack([(p <= f - 128 * j) for j in range(NQC)], axis=1)
    import ml_dtypes
    return m.astype(ml_dtypes.bfloat16)


def make_in_maps(x, w_attn, b_attn, w_proj):
    import ml_dtypes
    bf16 = ml_dtypes.bfloat16
    xT = np.ascontiguousarray(
        np.transpose(x, (0, 2, 1)).astype(bf16))
    mask = make_mask()
    in_maps = []
    for c in range(N_CORES):
        cs = slice(CW * c, CW * (c + 1))
        in_maps.append({
            "xT": xT,
            "wq": np.ascontiguousarray(w_attn[:, 0 * D:1 * D][:, cs]).astype(bf16),
            "wk": np.ascontiguousarray(w_attn[:, 1 * D:2 * D][:, cs]).astype(bf16),
            "wv": np.ascontiguousarray(w_attn[:, 2 * D:3 * D][:, cs]).astype(bf16),
            "bq": np.ascontiguousarray(b_attn[0 * D:1 * D][cs]) * 0.125,
            "bk": np.ascontiguousarray(b_attn[1 * D:2 * D][cs]),
            "wp": np.ascontiguousarray(w_proj[cs, :]).astype(bf16),
            "mask": mask,
        })
    return in_maps


def host_bias(b_attn, b_proj, w_proj):
    # v-bias propagates exactly through softmax (rows sum to 1) and the linear
    # projection: out += b_v @ w_proj + b_proj
    return b_proj.astype(np.float32) + b_attn[2 * D:3 * D].astype(np.float32) @ w_proj.astype(np.float32)


_NC_CACHE = {}


def get_program(reps: int = 1, phases: str = "123"):
    key = (reps, phases)
    if key not in _NC_CACHE:
        _NC_CACHE[key] = build_program(reps, phases)
    return _NC_CACHE[key]


def kernel(x, w_attn, b_attn, w_proj, b_proj):
    x = np.asarray(x, np.float32)
    w_attn = np.asarray(w_attn, np.float32)
    b_attn = np.asarray(b_attn, np.float32)
    w_proj = np.asarray(w_proj, np.float32)
    b_proj = np.asarray(b_proj, np.float32)

    nc = get_program()
    in_maps = make_in_maps(x, w_attn, b_attn, w_proj)
    res = run_bass_kernel_spmd(nc, in_maps, core_ids=list(range(N_CORES)))
    acc = np.zeros((B, T, D), np.float64)
    for r in res.results:
        acc += r["out"].astype(np.float64)
    acc += host_bias(b_attn, b_proj, w_proj).astype(np.float64)
    return acc.astype(np.float32)



# revision 62
# speedup vs baseline: 1.1188x; 1.1188x over previous
"""Causal multi-head attention block (B=2, T=2048, D=1024, H=16) on 8 TRN2 cores.

Sharding: tensor-parallel over heads — each core owns 2 heads (128 cols of
w_attn's q/k/v blocks, 128 rows of w_proj) and produces a partial output
[B, T, D]; the host sums the 8 partials and adds the bias terms.

Per-core kernel. All matmul operands are bf16 (fp32 psum accumulation): bf16
runs 1 cycle/row at ANY output width, whereas fp32r drops to 4 cycles/row
below 256-wide — which crippled the 256 [*,128]-wide v matmuls (~40us/iter).

  phase 1 (QKV):  qT,kT [128f, B*T] = w^T @ x^T   (w stationary, xT moving)
                  v     [B*T, 128]  = x @ w_v     (xT tiles stationary)
                  v stored per (t-tile, head) as [128, 65] with a ones column
                  LAST — the ones column makes the AV matmul also produce the
                  softmax denominator in psum partition 64.
                  x t-chunks 0-1 come from persistent tiles prefetched at the
                  PREVIOUS iteration's tail (sync queue, ahead of the last out
                  DMAs) so the loop boundary never stalls the PE ~10us on x.
  phase 2 (attn): per (b, q-chunk of 512), both heads together:
                  sT [128k, 512q] = k @ qT (scores, transposed layout; causal
                  block-skipping: only k-tiles <= q-chunk get computed)
                  probs = exp(sT) (one flat [128,1024] ACT op for both heads),
                  straddle blocks get multiplicative {0,1} mask strips (DVE)
                  avT [65, 512] += [v|ones]^T @ probs  (row 64 = sum of exp)
                  normalize: DVE copy of the psum denominator row -> DMA shift
                  to partition 0 (gpsimd) -> reciprocal_approx_fast (DVE, ~5x
                  cheaper than exact reciprocal) -> partition_broadcast ->
                  aT = avT[0:64] * bc  -> a_sb (heads stacked via gpsimd DMA)
  phase 3 (proj): out [128t, 512e] = a_sb^T(128=2 heads) @ w_proj rows.
                  Emitted one q-chunk LATE (after the next chunk's attention)
                  so the PE rides over the ~4us normalize-chain latency; the
                  final chunk reads the per-head at tiles directly with
                  half-depth matmuls. Out rows are assembled to [128, 1024]
                  bf16 in SBUF and written with one DMA per t-tile.
"""
import numpy as np

import concourse.bass as bass
import concourse.mybir as mybir
import concourse.tile as tile
from concourse import bacc
from concourse.bass import ts, ds
from concourse.bass_utils import run_bass_kernel_spmd

F32 = mybir.dt.float32
F32R = mybir.dt.float32r
BF16 = mybir.dt.bfloat16

B, T, D = 2, 2048, 1024
H = 16
HD = D // H          # 64
N_CORES = 8
HPC = H // N_CORES   # heads per core = 2
CW = HPC * HD        # per-core head width = 128
TCH = 512            # q/t chunk width
NTCH = (B * T) // TCH   # 8 t-chunks over flattened (b, t)
NKT = T // 128       # 16 k-tiles per batch
NQC = T // TCH       # 4 q-chunks per batch


def build_program(reps: int = 1, phases: str = "123"):
    """Build the per-core Bass program (same program on all 8 cores).

    reps>1 wraps the body in a dynamic loop for wall-clock timing runs.
    """
    nc = bacc.Bacc("TRN2", target_bir_lowering=False, debug=False,
                   num_devices=N_CORES)

    xT = nc.dram_tensor("xT", [B, D, T], BF16, kind="ExternalInput")
    wq = nc.dram_tensor("wq", [D, CW], BF16, kind="ExternalInput")
    wk = nc.dram_tensor("wk", [D, CW], BF16, kind="ExternalInput")
    wv = nc.dram_tensor("wv", [D, CW], BF16, kind="ExternalInput")
    bq = nc.dram_tensor("bq", [CW], F32, kind="ExternalInput")   # pre-scaled /8
    bk = nc.dram_tensor("bk", [CW], F32, kind="ExternalInput")
    wp = nc.dram_tensor("wp", [CW, D], BF16, kind="ExternalInput")
    mask = nc.dram_tensor("mask", [128, NQC, TCH], BF16, kind="ExternalInput")
    out = nc.dram_tensor("out", [B, T, D], BF16, kind="ExternalOutput")

    with tile.TileContext(nc) as tc:
        with (
            tc.tile_pool(name="const", bufs=1) as const,
            tc.tile_pool(name="persist", bufs=1) as persist,
            tc.tile_pool(name="xt", bufs=4) as xt_pool,
            tc.tile_pool(name="probs", bufs=8) as probs_pool,
            tc.tile_pool(name="norm", bufs=3) as norm_pool,
            tc.tile_pool(name="osb", bufs=4) as osb_pool,
        ):
            # ---- constants / persistent state ----
            # issue-order matters at kernel start: the first QKV matmuls need
            # wq/wk and the first x d-tiles; everything else can trail.
            wq_sb = const.tile([128, D // 128, CW], BF16)
            wk_sb = const.tile([128, D // 128, CW], BF16)
            wv_sb = const.tile([128, D // 128, CW], BF16)
            wp_sb = const.tile([128, D], BF16)
            bq_sb = const.tile([128, 1], F32)
            bk_sb = const.tile([128, 1], F32)
            mask_sb = const.tile([128, NQC, TCH], BF16)
            # consts go on the gpsimd DMA queue so the first x chunks (sync
            # queue) land in parallel
            wq_r = wq.rearrange("(dt p) m -> p dt m", p=128)
            wk_r = wk.rearrange("(dt p) m -> p dt m", p=128)
            nc.gpsimd.dma_start(wq_sb[:, 0:1, :], wq_r[:, 0:1, :])
            nc.gpsimd.dma_start(wk_sb[:, 0:1, :], wk_r[:, 0:1, :])
            nc.gpsimd.dma_start(wq_sb[:, 1:, :], wq_r[:, 1:, :])
            nc.gpsimd.dma_start(wk_sb[:, 1:, :], wk_r[:, 1:, :])
            nc.gpsimd.dma_start(wv_sb[:], wv.rearrange("(dt p) m -> p dt m", p=128))
            nc.gpsimd.dma_start(bq_sb[:], bq[:, None])
            nc.gpsimd.dma_start(bk_sb[:], bk[:, None])
            nc.gpsimd.dma_start(wp_sb[:], wp[:, :])
            wp2_sb = const.tile([HD, HPC, D], BF16)
            nc.gpsimd.dma_start(wp2_sb[:], wp.rearrange("(h d) e -> d h e", h=HPC))
            nc.gpsimd.dma_start(mask_sb[:], mask[:, :, :])

            qT_sb = persist.tile([128, B * T], BF16)   # [2h*64, (b,t)]
            kT_sb = persist.tile([128, B * T], BF16)
            a_sb = persist.tile([128, B * T], BF16)    # normalized attn out ^T
            # persistent x tiles for t-chunks 0-1: loaded in the prologue and
            # re-loaded at each iteration's tail (the sync queue is busy with
            # out-DMAs there, but the transfers overlap the projection tail),
            # so the loop boundary never stalls the PE on the first x chunks.
            px = [persist.tile([128, D // 128, TCH], BF16, name=f"px{_c}")
                  for _c in range(2)]

            def emit_px_prefetch():
                for c in range(2):
                    b_i, qc = divmod(c, NQC)
                    nc.sync.dma_start(
                        px[c][:],
                        xT[b_i].rearrange("(dt p) t -> p dt t", p=128)[
                            :, :, ds(qc * TCH, TCH)])

            emit_px_prefetch()
            # v per t-tile & head: [v(64) | ones] columns — the trailing ones
            # column makes the AV matmul emit the softmax denominator in
            # psum partition 64 (64-aligned, so DVE ops can touch it).
            v_sb = persist.tile([128, B * T // 128, HPC, HD + 1], BF16)
            nc.vector.memset(v_sb[:, :, :, HD], 1.0)

            def body(_=None):
                # ================= phase 1: QKV projections =================
                with (
                    tc.tile_pool(name="ps_qk", bufs=4, space="PSUM") as ps_qk,
                    tc.tile_pool(name="ps_v", bufs=2, space="PSUM") as ps_v,
                ):
                    for c in range(NTCH):
                        b_i, qc = divmod(c, NQC)
                        if c < 2:
                            xt = px[c]   # prefetched at the previous tail
                        else:
                            xt = xt_pool.tile([128, D // 128, TCH], BF16)
                            xsrc = xT[b_i].rearrange(
                                "(dt p) t -> p dt t", p=128)[
                                :, :, ds(qc * TCH, TCH)]
                            # split per pair of d-tiles so the first matmul
                            # can start as soon as its slice lands
                            for g in range((D // 128) // 2):
                                nc.sync.dma_start(xt[:, ts(g, 2), :],
                                                  xsrc[:, ts(g, 2), :])
                        q_ps = ps_qk.tile([128, TCH], F32, tag="qk")
                        k_ps = ps_qk.tile([128, TCH], F32, tag="qk")
                        v_ps = ps_v.tile([128, TCH], F32)
                        # single-bank accumulation runs (all q, then all
                        # k) — avoids per-mm psum bank ping-pong; the q run
                        # still starts as the first DMA slices land
                        nd = D // 128
                        for dt in range(nd):
                            nc.tensor.matmul(q_ps[:], wq_sb[:, dt, :],
                                             xt[:, dt, :], start=dt == 0,
                                             stop=dt == nd - 1)
                        for dt in range(nd):
                            nc.tensor.matmul(k_ps[:], wk_sb[:, dt, :],
                                             xt[:, dt, :], start=dt == 0,
                                             stop=dt == nd - 1)
                        for s in range(TCH // 128):
                            for dt in range(nd):
                                nc.tensor.matmul(
                                    v_ps[:, ts(s, 128)],
                                    xt[:, dt, ts(s, 128)],
                                    wv_sb[:, dt, :],
                                    start=(dt == 0), stop=(dt == nd - 1))
                        # epilogues: q = psum/8 + bq/8 ; k = psum + bk
                        nc.vector.tensor_scalar(
                            qT_sb[:, ds(c * TCH, TCH)], q_ps[:], 0.125,
                            bq_sb[:],
                            mybir.AluOpType.mult, mybir.AluOpType.add)
                        nc.vector.tensor_scalar_add(
                            kT_sb[:, ds(c * TCH, TCH)], k_ps[:], bk_sb[:])
                        for s in range(TCH // 128):
                            tt = c * (TCH // 128) + s
                            nc.any.tensor_copy(
                                v_sb[:, tt, :, 0:HD],
                                v_ps[:, ds(s * 128, 128)].rearrange(
                                    "p (h d) -> p h d", h=HPC))

                if "2" not in phases:
                    return
                # ========== phases 2+3: attention + projection ==============
                # both heads processed together per (batch, q-chunk): their
                # 64-contraction score matmuls sit on partition bases 0/64 so
                # the PE runs them concurrently (row groups), and one ACT exp
                # covers both heads.
                with (
                    tc.tile_pool(name="ps_s", bufs=2, space="PSUM") as ps_s,
                    tc.tile_pool(name="ps_av", bufs=3, space="PSUM") as ps_av,
                    tc.tile_pool(name="ps_o", bufs=1, space="PSUM") as ps_o,
                ):
                    def emit_scores(b_i, qc, kt):
                        """scores + exp (+mask) for one k-block; returns the
                        probs tile for the AV matmuls."""
                        j = kt - 4 * qc   # >=0: straddles diagonal
                        f0 = max(j, 0) * 128  # cols f<128j fully masked
                        fsl = ds(f0, TCH - f0)
                        qcol = ds(b_i * T + qc * TCH, TCH)
                        ktcol = ds(b_i * T + kt * 128, 128)
                        sp_ps = ps_s.tile([128, HPC, TCH], F32)
                        for h in range(HPC):
                            hp = ds(h * HD, HD)
                            nc.tensor.matmul(
                                sp_ps[:, h, fsl],
                                kT_sb[hp, ktcol],
                                qT_sb[hp, qcol][:, fsl],
                                start=True, stop=True)
                        pp_sb = probs_pool.tile([128, HPC, TCH], BF16)
                        # full-width pairs get one flat contiguous [128, 1024]
                        # exp (the fast ACT shape); sliced straddle tiles get
                        # two contiguous 2D ops
                        if f0 == 0:
                            nc.scalar.activation(
                                pp_sb.rearrange("p h w -> p (h w)"),
                                sp_ps.rearrange("p h w -> p (h w)"),
                                mybir.ActivationFunctionType.Exp)
                        else:
                            for h in range(HPC):
                                nc.scalar.activation(
                                    pp_sb[:, h, fsl], sp_ps[:, h, fsl],
                                    mybir.ActivationFunctionType.Exp)
                        if j >= 0:
                            # only the 128-wide DIAGONAL q-subtile needs
                            # masking: for q-subcolumns beyond it the keep
                            # condition (p <= f - 128j) is always true, so
                            # multiplying there was a no-op. 128-wide ops cut
                            # DVE mask work ~2.5x and shrink the mask latency
                            # that gates the AV matmul behind the DVE backlog.
                            dsl = ds(f0, 128)
                            for h in range(HPC):
                                nc.vector.tensor_tensor(
                                    pp_sb[:, h, dsl], pp_sb[:, h, dsl],
                                    mask_sb[:, j, dsl],
                                    mybir.AluOpType.mult)
                        return pp_sb, fsl

                    def emit_av(b_i, qc, kt, avs, pp_sb, fsl):
                        nkt = 4 * qc + 4
                        for h in range(HPC):
                            nc.tensor.matmul(
                                avs[h][:, fsl],
                                v_sb[:, b_i * NKT + kt, h, :],
                                pp_sb[:, h, fsl],
                                start=(kt == 0), stop=(kt == nkt - 1))

                    def emit_attn(b_i, qc, last):
                        """attention for one q-chunk, then the softmax
                        normalize chain. Returns per-head at tiles."""
                        nkt = 4 * qc + 4   # causal: k-tiles 0..nkt-1
                        avs = [ps_av.tile([HD + 1, TCH], F32, tag="av",
                                          name=f"av{_h}")
                               for _h in range(HPC)]
                        for kt in range(nkt):
                            pp = emit_scores(b_i, qc, kt)
                            emit_av(b_i, qc, kt, avs, *pp)
                        # normalize rows 0..63 by the ones-row 64
                        qcol = ds(b_i * T + qc * TCH, TCH)
                        at_tiles = []
                        for h in range(HPC):
                            hp = ds(h * HD, HD)
                            av_ps = avs[h]
                            # denominator row: PSUM -> SBUF copy (DMA and the
                            # approx-recip bit trick can't touch PSUM), then
                            # DMA-shift to partition 0 for partition_broadcast
                            # (gpsimd queue: the pbcast consumer is next there
                            # anyway).
                            d64_sb = norm_pool.tile([HD + 1, TCH], F32,
                                                    tag="d64")
                            nc.vector.tensor_copy(
                                d64_sb[HD:HD + 1, :], av_ps[HD:HD + 1, :])
                            d0_sb = norm_pool.tile([1, TCH], F32, tag="d0")
                            nc.gpsimd.dma_start(d0_sb[:],
                                                d64_sb[HD:HD + 1, :])
                            r0_sb = norm_pool.tile([1, TCH], F32, tag="r0")
                            nc.vector.reciprocal_approx_fast(
                                r0_sb[:], d0_sb[:])
                            bc_sb = norm_pool.tile([HD, TCH], F32, tag="bc")
                            nc.gpsimd.partition_broadcast(bc_sb[:], r0_sb[:])
                            at_sb = norm_pool.tile([HD, TCH], BF16, tag="at")
                            nc.vector.tensor_tensor(
                                at_sb[:], av_ps[0:HD, :], bc_sb[:],
                                mybir.AluOpType.mult)
                            at_tiles.append(at_sb)
                            if not last:
                                # partition-shift into stacked-head layout
                                nc.gpsimd.dma_start(a_sb[hp, qcol], at_sb[:])
                        return at_tiles

                    def emit_proj(b_i, qc, at_tiles, last):
                        """projection for one q-chunk. Non-last chunks are
                        emitted one chunk late (after the NEXT chunk's
                        attention) so the PE rides over the normalize-chain
                        latency. The very last chunk reads the per-head at
                        tiles directly (half-depth matmuls) and borrows av-tag
                        psum slots to pipeline the copies."""
                        for ti, tt in enumerate(range(qc * 4, qc * 4 + 4)):
                            o_sb = osb_pool.tile([128, D], BF16)
                            for ec in range(D // TCH):
                                i = ti * (D // TCH) + ec
                                # last chunk: rotate through FOUR banks (3
                                # borrowed av slots + o) so each matmul never
                                # waits a cast latency — this drain is what
                                # the next iteration's QKV weights queue
                                # behind (~6.5us boundary stall at 2 banks)
                                if last and i % 4 != 0:
                                    o_ps = ps_av.tile([128, TCH], F32,
                                                      tag="av", name="o_av")
                                else:
                                    o_ps = ps_o.tile([128, TCH], F32)
                                if last:
                                    tloc = ds((tt - qc * 4) * 128, 128)
                                    for h in range(HPC):
                                        nc.tensor.matmul(
                                            o_ps[:],
                                            at_tiles[h][:, tloc],
                                            wp2_sb[:, h, ts(ec, TCH)],
                                            start=(h == 0),
                                            stop=(h == HPC - 1))
                                else:
                                    nc.tensor.matmul(
                                        o_ps[:],
                                        a_sb[:, ds(b_i * T + tt * 128, 128)],
                                        wp_sb[:, ts(ec, TCH)],
                                        start=True, stop=True)
                                nc.any.tensor_copy(
                                    o_sb[:, ts(ec, TCH)], o_ps[:])
                            nc.sync.dma_start(
                                out[b_i, ts(tt, 128), :], o_sb[:])

                    pending = None
                    for b_i in range(B):
                        for qc in range(NQC):
                            last = (b_i == B - 1 and qc == NQC - 1)
                            at_tiles = emit_attn(b_i, qc, last)
                            if "3" not in phases:
                                continue
                            if b_i == B - 1 and qc == 0:
                                # re-load the first x chunks for the next
                                # iteration. Emitted HERE (mid phase 2, ~50us
                                # before the tail) so the 2MB of transfers
                                # never competes with the tail projections'
                                # out-DMAs — issued at the tail it delayed
                                # o_sb recycling and stalled the next
                                # iteration's PE ~6.5us through the weight
                                # shadow.
                                emit_px_prefetch()
                            if pending is not None:
                                emit_proj(*pending, last=False)
                            if last:
                                emit_proj(b_i, qc, at_tiles, last=True)
                            else:
                                pending = (b_i, qc, None)

            if reps == 1:
                body()
            else:
                with tc.For_i(0, reps, 1) as _i:
                    body(_i)

    nc.compile()
    return nc


def make_mask() -> np.ndarray:
    """Multiplicative mask strips for the 4 diagonal-straddling k-tiles of a
    512-wide q-chunk: strip j keeps (p <= f - 128*j)."""
    p = np.arange(128)[:, None]
    f = np.arange(TCH)[None, :]
    m = np.stack([(p <= f - 128 * j) for j in range(NQC)], axis=1)
    import ml_dtypes
    return m.astype(ml_dtypes.bfloat16)


def make_in_maps(x, w_attn, b_attn, w_proj):
    import ml_dtypes
    bf16 = ml_dtypes.bfloat16
    xT = np.ascontiguousarray(
        np.transpose(x, (0, 2, 1)).astype(bf16))
    mask = make_mask()
    in_maps = []
    for c in range(N_CORES):
        cs = slice(CW * c, CW * (c + 1))
        in_maps.append({
            "xT": xT,
            "wq": np.ascontiguousarray(w_attn[:, 0 * D:1 * D][:, cs]).astype(bf16),
            "wk": np.ascontiguousarray(w_attn[:, 1 * D:2 * D][:, cs]).astype(bf16),
            "wv": np.ascontiguousarray(w_attn[:, 2 * D:3 * D][:, cs]).astype(bf16),
            "bq": np.ascontiguousarray(b_attn[0 * D:1 * D][cs]) * 0.125,
            "bk": np.ascontiguousarray(b_attn[1 * D:2 * D][cs]),
            "wp": np.ascontiguousarray(w_proj[cs, :]).astype(bf16),
            "mask": mask,
        })
    return in_maps


def host_bias(b_attn, b_proj, w_proj):
    # v-bias propagates exactly through softmax (rows sum to 1) and the linear
    # projection: out += b_v @ w_proj + b_proj
    return b_proj.astype(np.float32) + b_attn[2 * D:3 * D].astype(np.float32) @ w_proj.astype(np.float32)


_NC_CACHE = {}


def get_program(reps: int = 1, phases: str = "123"):
    key = (reps, phases)
    if key not in _NC_CACHE:
        _NC_CACHE[key] = build_program(reps, phases)
    return _NC_CACHE[key]


def kernel(x, w_attn, b_attn, w_proj, b_proj):
    x = np.asarray(x, np.float32)
    w_attn = np.asarray(w_attn, np.float32)
    b_attn = np.asarray(b_attn, np.float32)
    w_proj = np.asarray(w_proj, np.float32)
    b_proj = np.asarray(b_proj, np.float32)

    nc = get_program()
    in_maps = make_in_maps(x, w_attn, b_attn, w_proj)
    res = run_bass_kernel_spmd(nc, in_maps, core_ids=list(range(N_CORES)))
    acc = np.zeros((B, T, D), np.float64)
    for r in res.results:
        acc += r["out"].astype(np.float64)
    acc += host_bias(b_attn, b_proj, w_proj).astype(np.float64)
    return acc.astype(np.float32)

